# revision 4
# baseline (speedup 1.0000x reference)
"""Causal single-head attention on 8 trn2 NeuronCores, data-parallel over batch.

Per core (one batch element, C=2048 ctx, E=1024 emb, D=1024 query_dim):

  P_X: DMA x row-tiles (fp32), cast to fp16 on GpSimd, DMA-XBAR-transpose
       into resident xt [e=128, C] fp16 tiles.  Zero PE cost.
  P_Q/P_K: qT/kT = (W^T @ x^T) + b via PE matmuls (N=512 moving), bias
       fused in the scalar-engine PSUM->SBUF copy.  Resident fp16.
  P_V: v = x @ Vw natural layout; bias added by DVE (broadcast tile vbb)
       during the PSUM->SBUF copy.  Resident fp16.
  Attention (software-pipelined by one row-block):
       S_i: scores for 128-query block i (PE, causal chunks of 512),
            additive mask on the diagonal tile (DVE), exp on ACT with
            fused row-sum accumulation.
       O_{i-1}: E^T via DMA-XBAR-transpose (no PE), AV accumulation on
            PE, 1/rowsum scale on DVE, DMA out.
       PE stream: S_0 S_1 O_0 S_2 O_1 ... S_15 O_14 O_15 -- the exp and
       transposes of block i hide under S_{i+1}/O_{i-1} PE work.

Loop mode (benchmarking): `loop` bodies total, emitted as For_i over
loop/unroll trips with `unroll` bodies per trip.  Unrolled bodies
overlap point-to-point (DMA/cast prefetch of body u+1 during body u's
attention); the all-engine For_i barrier is paid once per trip.
"""

import os
import sys

for _p in ("/opt/trn_rl_repo", "/root/.axon_site/_ro/trn_rl_repo"):
    if os.path.isdir(_p) and _p not in sys.path:
        sys.path.insert(0, _p)

from contextlib import ExitStack

import numpy as np

import concourse.bass as bass
import concourse.tile as tile
from concourse import bacc, mybir
from concourse.masks import make_causal_mask, make_identity

F32 = mybir.dt.float32
AF = mybir.ActivationFunctionType
DTYPES = {"fp16": mybir.dt.float16, "bf16": mybir.dt.bfloat16}

P = 128


def build(C=2048, E=1024, D=1024, n_cores=8, loop=1, dt="fp16", unroll=1,
          xt_dma=True, et_dma=True):
    DT = DTYPES[dt]
    CC = 512            # c-chunk width for projection passes
    NJ = 512            # matmul moving-dim width
    NCC = C // CC
    EC = E // P         # contraction chunks for projections
    DC = D // P
    RB = C // P         # number of 128-row blocks
    ND = D // NJ
    scale = float(D) ** -0.5

    assert loop % unroll == 0 or loop == 1
    nc = bacc.Bacc("TRN2", target_bir_lowering=False, debug=False,
                   num_devices=n_cores)
    x_d = nc.dram_tensor("x", [C, E], F32, kind="ExternalInput").ap()
    qw_d = nc.dram_tensor("Qw", [E, D], F32, kind="ExternalInput").ap()
    qb_d = nc.dram_tensor("Qb", [D], F32, kind="ExternalInput").ap()
    kw_d = nc.dram_tensor("Kw", [E, D], F32, kind="ExternalInput").ap()
    kb_d = nc.dram_tensor("Kb", [D], F32, kind="ExternalInput").ap()
    vw_d = nc.dram_tensor("Vw", [E, D], F32, kind="ExternalInput").ap()
    vb_d = nc.dram_tensor("Vb", [D], F32, kind="ExternalInput").ap()
    out_d = nc.dram_tensor("out", [C, D], F32, kind="ExternalOutput").ap()

    with tile.TileContext(nc) as tc, ExitStack() as ctx:
        # ---- loop-invariant constants (outside the For_i loop)
        const_pool = ctx.enter_context(tc.tile_pool(name="const", bufs=1))
        ident_f = const_pool.tile([P, P], F32, name="ident_f")
        make_identity(nc, ident_f)
        ident_h = const_pool.tile([P, P], DT, name="ident_h")
        nc.vector.tensor_copy(ident_h[:], ident_f[:])
        cmask = const_pool.tile([P, P], F32, name="cmask")
        make_causal_mask(nc, cmask, mask_val=-1e9)
        qb_t = const_pool.tile([P, DC], F32, name="qb_t")
        nc.sync.dma_start(qb_t[:], qb_d.rearrange("(c p) -> p c", p=P))
        kb_t = const_pool.tile([P, DC], F32, name="kb_t")
        nc.sync.dma_start(kb_t[:], kb_d.rearrange("(c p) -> p c", p=P))
        # V bias broadcast to all partitions (DVE adds it during P_V copy)
        vb_f = const_pool.tile([1, D], F32, name="vb_f")
        nc.sync.dma_start(vb_f[:], vb_d[None, :])
        vbb = const_pool.tile([P, D], F32, name="vbb")
        nc.gpsimd.partition_broadcast(vbb[:], vb_f[:], channels=P)

        # ---- persistent per-body tensors (shared across unrolled bodies;
        # WAR deps give natural cross-body pipelining)
        xt_pool = ctx.enter_context(tc.tile_pool(name="xt", bufs=1))
        qt_pool = ctx.enter_context(tc.tile_pool(name="qt", bufs=1))
        kt_pool = ctx.enter_context(tc.tile_pool(name="kt", bufs=1))
        v_pool = ctx.enter_context(tc.tile_pool(name="v", bufs=1))
        xt = [xt_pool.tile([P, C], DT, name=f"xt{e}") for e in range(EC)]
        qt_sb = [qt_pool.tile([P, C], DT, name=f"qt{d}") for d in range(DC)]
        kt_sb = [kt_pool.tile([P, C], DT, name=f"kt{d}") for d in range(DC)]
        v_sb = [v_pool.tile([P, D], DT, name=f"v{i}") for i in range(RB)]

        # staging pools, persistent so WAR rotation spans bodies
        xst_pool = ctx.enter_context(tc.tile_pool(name="xst", bufs=2))
        xh_pool = ctx.enter_context(tc.tile_pool(name="xh", bufs=4))
        wst_pool = ctx.enter_context(tc.tile_pool(name="wst", bufs=2))
        w_pool = ctx.enter_context(tc.tile_pool(name="w", bufs=1))
        w_sb = [w_pool.tile([P, D], DT, name=f"w{e}") for e in range(EC)]
        e_pool = ctx.enter_context(tc.tile_pool(name="e", bufs=2))
        et_pool = ctx.enter_context(tc.tile_pool(name="et", bufs=2))
        r_pool = ctx.enter_context(tc.tile_pool(name="r", bufs=4))
        os_pool = ctx.enter_context(tc.tile_pool(name="os", bufs=2))

        def emit_body():
            # ---- P_X: x -> fp16 -> xt (transposed) without touching PE
            with tc.tile_pool(name="px_ps", bufs=4, space="PSUM") \
                    if not xt_dma else ExitStack() as px_pool:
                for ct in range(RB):
                    xst = xst_pool.tile([P, E], F32, tag="xst")
                    nc.sync.dma_start(
                        xst[:], x_d[ct * P:(ct + 1) * P, :])
                    xh = xh_pool.tile([P, E], DT, tag="xh")
                    nc.gpsimd.tensor_copy(xh[:], xst[:])
                    if xt_dma:
                        for e in range(EC):
                            nc.sync.dma_start(
                                xt[e][:, ct * P:(ct + 1) * P],
                                xh[:, e * P:(e + 1) * P], transpose=True)
                    else:
                        for e in range(EC):
                            ps_x = px_pool.tile([P, P], DT, tag="px")
                            nc.tensor.transpose(
                                ps_x[:], xh[:, e * P:(e + 1) * P], ident_h[:])
                            nc.vector.tensor_copy(
                                xt[e][:, ct * P:(ct + 1) * P], ps_x[:])

            def load_w(w_dram):
                for e in range(EC):
                    wst = wst_pool.tile([P, D], F32, tag="wst")
                    nc.sync.dma_start(wst[:], w_dram[e * P:(e + 1) * P, :])
                    nc.vector.tensor_copy(w_sb[e][:], wst[:])

            def proj_t(bias_t, dest, ps_pool):
                for cc in range(NCC):
                    for dc in range(DC):
                        ps = ps_pool.tile([P, CC], F32, tag="ps")
                        for e in range(EC):
                            nc.tensor.matmul(
                                ps[:],
                                w_sb[e][:, dc * P:(dc + 1) * P],
                                xt[e][:, cc * CC:(cc + 1) * CC],
                                start=(e == 0), stop=(e == EC - 1))
                        nc.scalar.activation(
                            dest[dc][:, cc * CC:(cc + 1) * CC], ps[:],
                            AF.Identity, bias=bias_t[:, dc:dc + 1])

            with tc.tile_pool(name="pp_ps", bufs=6, space="PSUM") as ps_pool:
                load_w(qw_d)
                proj_t(qb_t, qt_sb, ps_pool)
                load_w(kw_d)
                proj_t(kb_t, kt_sb, ps_pool)
                # ---- P_V: natural layout, bias via DVE add
                load_w(vw_d)
                for ct in range(RB):
                    for dh in range(ND):
                        ps = ps_pool.tile([P, NJ], F32, tag="ps")
                        for e in range(EC):
                            nc.tensor.matmul(
                                ps[:], xt[e][:, ct * P:(ct + 1) * P],
                                w_sb[e][:, dh * NJ:(dh + 1) * NJ],
                                start=(e == 0), stop=(e == EC - 1))
                        nc.vector.tensor_add(
                            v_sb[ct][:, dh * NJ:(dh + 1) * NJ], ps[:],
                            vbb[:, dh * NJ:(dh + 1) * NJ])

            # ---- attention, software-pipelined by one row-block
            with tc.tile_pool(name="a_s", bufs=5, space="PSUM") as s_pool, \
                 tc.tile_pool(name="a_t", bufs=2, space="PSUM") as t_pool, \
                 tc.tile_pool(name="a_o", bufs=2, space="PSUM") as o_pool:

                def s_chain(i):
                    ncols = (i + 1) * P
                    njj = (ncols + NJ - 1) // NJ
                    etile = e_pool.tile([P, C], DT, tag="E")
                    acc = r_pool.tile([P, NCC], F32, tag="acc")
                    for jj in range(njj):
                        n = min(NJ, ncols - jj * NJ)
                        ps_s = s_pool.tile([P, NJ], F32, tag="s")
                        for d in range(DC):
                            nc.tensor.matmul(
                                ps_s[:, :n],
                                qt_sb[d][:, i * P:(i + 1) * P],
                                kt_sb[d][:, jj * NJ:jj * NJ + n],
                                start=(d == 0), stop=(d == DC - 1))
                        if jj == njj - 1:
                            dcol = n - P
                            nc.vector.tensor_add(ps_s[:, dcol:dcol + P],
                                                 ps_s[:, dcol:dcol + P],
                                                 cmask[:])
                        nc.scalar.activation(
                            etile[:, jj * NJ:jj * NJ + n], ps_s[:, :n],
                            AF.Exp, scale=scale,
                            accum_out=acc[:, jj:jj + 1])
                    return etile, acc

                def o_chain(k, etile, acc):
                    ncols = (k + 1) * P
                    njj = (ncols + NJ - 1) // NJ
                    rs = r_pool.tile([P, 1], F32, tag="rs")
                    nc.vector.reduce_sum(rs[:], acc[:, :njj],
                                         axis=mybir.AxisListType.X)
                    rinv = r_pool.tile([P, 1], F32, tag="rinv")
                    nc.vector.reciprocal(rinv[:], rs[:])

                    ettile = et_pool.tile([P, C], DT, tag="ET")
                    if et_dma:
                        for b in range(k + 1):
                            nc.scalar.dma_start(
                                ettile[:, b * P:(b + 1) * P],
                                etile[:, b * P:(b + 1) * P], transpose=True)
                    else:
                        for jj in range(njj):
                            n = min(NJ, ncols - jj * NJ)
                            ps_t = t_pool.tile([P, NJ], DT, tag="t")
                            for b in range(n // P):
                                nc.tensor.transpose(
                                    ps_t[:, b * P:(b + 1) * P],
                                    etile[:, jj * NJ + b * P:
                                          jj * NJ + (b + 1) * P],
                                    ident_h[:])
                            nc.vector.tensor_copy(
                                ettile[:, jj * NJ:jj * NJ + n], ps_t[:, :n])

                    outst = os_pool.tile([P, D], F32, tag="os")
                    for dh in range(ND):
                        ps_o = o_pool.tile([P, NJ], F32, tag="o")
                        for j in range(k + 1):
                            nc.tensor.matmul(
                                ps_o[:],
                                ettile[:, j * P:(j + 1) * P],
                                v_sb[j][:, dh * NJ:(dh + 1) * NJ],
                                start=(j == 0), stop=(j == k))
                        nc.vector.tensor_scalar_mul(
                            outst[:, dh * NJ:(dh + 1) * NJ], ps_o[:],
                            rinv[:])
                    nc.scalar.dma_start(out_d[k * P:(k + 1) * P, :],
                                        outst[:])

                prev = None
                for i in range(RB):
                    cur = s_chain(i)
                    if prev is not None:
                        o_chain(i - 1, *prev)
                    prev = cur
                o_chain(RB - 1, *prev)

        if loop > 1:
            with tc.For_i(0, loop // unroll, 1):
                for _u in range(unroll):
                    emit_body()
        else:
            emit_body()

    nc.compile()
    return nc


_CACHE = {}


def _built(C=2048, E=1024, D=1024, n_cores=8, loop=1, dt="fp16", unroll=1):
    xt_dma = os.environ.get("K_XT_DMA", "1") == "1"
    et_dma = os.environ.get("K_ET_DMA", "1") == "1"
    key = (C, E, D, n_cores, loop, dt, unroll, xt_dma, et_dma)
    if key not in _CACHE:
        _CACHE[key] = build(C, E, D, n_cores, loop, dt, unroll,
                            xt_dma=xt_dma, et_dma=et_dma)
    return _CACHE[key]


def _executable(C=2048, E=1024, D=1024, n_cores=8, loop=1, dt="fp16",
                unroll=1):
    """Cached jitted SPMD executable for the built Bass module."""
    key = ("exec", C, E, D, n_cores, loop, dt, unroll)
    if key in _CACHE:
        return _CACHE[key]
    import jax
    from jax.sharding import Mesh, PartitionSpec
    from jax.experimental.shard_map import shard_map
    from concourse import bass2jax, mybir as _mybir

    nc = _built(C, E, D, n_cores, loop, dt, unroll)
    bass2jax.install_neuronx_cc_hook()

    partition_name = (nc.partition_id_tensor.name
                      if nc.partition_id_tensor else None)
    in_names, out_names, out_avals, zero_outs = [], [], [], []
    for alloc in nc.m.functions[0].allocations:
        if not isinstance(alloc, _mybir.MemoryLocationSet):
            continue
        name = alloc.memorylocations[0].name
        if alloc.kind == "ExternalInput":
            if name != partition_name:
                in_names.append(name)
        elif alloc.kind == "ExternalOutput":
            out_names.append(name)
            shape = tuple(alloc.tensor_shape)
            dtype = _mybir.dt.np(alloc.dtype)
            out_avals.append(jax.core.ShapedArray(shape, dtype))
            zero_outs.append(np.zeros(shape, dtype))
    n_params = len(in_names)
    all_names = in_names + out_names
    if partition_name is not None:
        all_names = all_names + [partition_name]

    def _body(*args):
        operands = list(args)
        if partition_name is not None:
            operands.append(bass2jax.partition_id_tensor())
        outs = bass2jax._bass_exec_p.bind(
            *operands,
            out_avals=tuple(out_avals),
            in_names=tuple(all_names),
            out_names=tuple(out_names),
            lowering_input_output_aliases=(),
            sim_require_finite=True,
            sim_require_nnan=True,
            nc=nc,
        )
        return tuple(outs)

    devices = jax.devices()[:n_cores]
    mesh = Mesh(np.asarray(devices), ("core",))
    n_outs = len(out_names)
    sharded = jax.jit(
        shard_map(_body, mesh=mesh,
                  in_specs=(PartitionSpec("core"),) * (n_params + n_outs),
                  out_specs=(PartitionSpec("core"),) * n_outs,
                  check_rep=False),
        donate_argnums=tuple(range(n_params, n_params + n_outs)),
        keep_unused=True,
    )
    res = dict(fn=sharded, in_names=in_names, out_names=out_names,
               out_avals=out_avals, zero_outs=zero_outs, mesh=mesh,
               n_cores=n_cores)
    _CACHE[key] = res
    return res


def run(inputs, C=2048, E=1024, D=1024, n_cores=8, dt="fp16"):
    ex = _executable(C, E, D, n_cores, 1, dt)
    B = inputs["x"].shape[0]
    assert B == n_cores
    f = lambda a: np.ascontiguousarray(np.asarray(a, dtype=np.float32))
    shared = {k: f(inputs[k]) for k in ("Qw", "Qb", "Kw", "Kb", "Vw", "Vb")}
    x = f(inputs["x"])
    per_core = [dict(x=x[b], **shared) for b in range(B)]
    concat_in = [
        np.concatenate([per_core[c][n] for c in range(n_cores)], axis=0)
        for n in ex["in_names"]
    ]
    concat_zeros = [
        np.zeros((n_cores * z.shape[0], *z.shape[1:]), z.dtype)
        for z in ex["zero_outs"]
    ]
    out_arrs = ex["fn"](*concat_in, *concat_zeros)
    i = ex["out_names"].index("out")
    out = np.asarray(out_arrs[i]).reshape(n_cores, *ex["out_avals"][i].shape)
    return out


def kernel(**inputs) -> np.ndarray:
    return run(inputs)


# revision 12
# speedup vs baseline: 1.3269x; 1.3269x over previous
"""Causal single-head attention on 8 trn2 NeuronCores, data-parallel over batch.

Per core (one batch element, C=2048 ctx, E=1024 emb, D=1024 query_dim):

  P_X: DMA x row-tiles (fp32), cast to fp16 on GpSimd, DMA-XBAR-transpose
       into resident xt [e=128, C] fp16 tiles.  Zero PE cost.
  P_Q/P_K: qT/kT = (W^T @ x^T) + b via PE matmuls (N=512 moving), bias
       fused in the scalar-engine PSUM->SBUF copy.  Resident fp16.
  P_V: v = x @ Vw natural layout; bias added by DVE (broadcast tile vbb)
       during the PSUM->SBUF copy.  Resident fp16.
  Attention (software-pipelined by one row-block):
       S_i: scores for 128-query block i (PE, causal chunks of 512),
            additive mask on the diagonal tile (DVE), exp on ACT with
            fused row-sum accumulation.
       O_{i-1}: E^T via DMA-XBAR-transpose (no PE), AV accumulation on
            PE, 1/rowsum scale on DVE, DMA out.
       PE stream: S_0 S_1 O_0 S_2 O_1 ... S_15 O_14 O_15 -- the exp and
       transposes of block i hide under S_{i+1}/O_{i-1} PE work.

Loop mode (benchmarking): `loop` bodies total, emitted as For_i over
loop/unroll trips with `unroll` bodies per trip.  Unrolled bodies
overlap point-to-point (DMA/cast prefetch of body u+1 during body u's
attention); the all-engine For_i barrier is paid once per trip.
"""

import os
import sys

for _p in ("/opt/trn_rl_repo", "/root/.axon_site/_ro/trn_rl_repo"):
    if os.path.isdir(_p) and _p not in sys.path:
        sys.path.insert(0, _p)

from contextlib import ExitStack

import numpy as np

import concourse.bass as bass
import concourse.tile as tile
from concourse import bacc, mybir
from concourse.masks import make_causal_mask, make_identity

F32 = mybir.dt.float32
AF = mybir.ActivationFunctionType
DTYPES = {"fp16": mybir.dt.float16, "bf16": mybir.dt.bfloat16}

P = 128


def build(C=2048, E=1024, D=1024, n_cores=8, loop=1, dt="fp16", unroll=1,
          xt_dma=True, et_dma=False):
    DT = DTYPES[dt]
    CC = 512            # c-chunk width for projection passes
    NJ = 512            # matmul moving-dim width
    NCC = C // CC
    EC = E // P         # contraction chunks for projections
    DC = D // P
    RB = C // P         # number of 128-row blocks
    ND = D // NJ
    scale = float(D) ** -0.5

    assert loop % unroll == 0 or loop == 1
    nc = bacc.Bacc("TRN2", target_bir_lowering=False, debug=False,
                   num_devices=n_cores)
    x_d = nc.dram_tensor("x", [C, E], F32, kind="ExternalInput").ap()
    qw_d = nc.dram_tensor("Qw", [E, D], F32, kind="ExternalInput").ap()
    qb_d = nc.dram_tensor("Qb", [D], F32, kind="ExternalInput").ap()
    kw_d = nc.dram_tensor("Kw", [E, D], F32, kind="ExternalInput").ap()
    kb_d = nc.dram_tensor("Kb", [D], F32, kind="ExternalInput").ap()
    vw_d = nc.dram_tensor("Vw", [E, D], F32, kind="ExternalInput").ap()
    vb_d = nc.dram_tensor("Vb", [D], F32, kind="ExternalInput").ap()
    out_d = nc.dram_tensor("out", [C, D], F32, kind="ExternalOutput").ap()

    with tile.TileContext(nc) as tc, ExitStack() as ctx:
        # ---- loop-invariant constants (outside the For_i loop)
        const_pool = ctx.enter_context(tc.tile_pool(name="const", bufs=1))
        ident_f = const_pool.tile([P, P], F32, name="ident_f")
        make_identity(nc, ident_f)
        ident_h = const_pool.tile([P, P], DT, name="ident_h")
        nc.vector.tensor_copy(ident_h[:], ident_f[:])
        cmask = const_pool.tile([P, P], F32, name="cmask")
        make_causal_mask(nc, cmask, mask_val=-1e9)
        qb_t = const_pool.tile([P, DC], F32, name="qb_t")
        nc.sync.dma_start(qb_t[:], qb_d.rearrange("(c p) -> p c", p=P))
        kb_t = const_pool.tile([P, DC], F32, name="kb_t")
        nc.sync.dma_start(kb_t[:], kb_d.rearrange("(c p) -> p c", p=P))
        # V bias broadcast to all partitions (DVE adds it during P_V copy)
        vb_f = const_pool.tile([1, D], F32, name="vb_f")
        nc.sync.dma_start(vb_f[:], vb_d[None, :])
        vbb = const_pool.tile([P, D], F32, name="vbb")
        nc.gpsimd.partition_broadcast(vbb[:], vb_f[:], channels=P)

        # ---- persistent per-body tensors (shared across unrolled bodies;
        # WAR deps give natural cross-body pipelining)
        xt_pool = ctx.enter_context(tc.tile_pool(name="xt", bufs=1))
        qt_pool = ctx.enter_context(tc.tile_pool(name="qt", bufs=1))
        kt_pool = ctx.enter_context(tc.tile_pool(name="kt", bufs=1))
        v_pool = ctx.enter_context(tc.tile_pool(name="v", bufs=1))
        # xt is one [P, EC*C] tile so one XBAR transpose per x row-tile can
        # scatter all EC chunks (3D out AP); chunk e lives at cols [e*C,(e+1)*C)
        xt_all = xt_pool.tile([P, EC * C], DT, name="xt")

        def xt_sl(e, lo, hi):
            return xt_all[:, e * C + lo:e * C + hi]

        qt_sb = [qt_pool.tile([P, C], DT, name=f"qt{d}") for d in range(DC)]
        kt_sb = [kt_pool.tile([P, C], DT, name=f"kt{d}") for d in range(DC)]
        v_sb = [v_pool.tile([P, D], DT, name=f"v{i}") for i in range(RB)]

        # staging pools, persistent so WAR rotation spans bodies
        xst_pool = ctx.enter_context(tc.tile_pool(name="xst", bufs=2))
        xh_pool = ctx.enter_context(tc.tile_pool(name="xh", bufs=2))
        wst_pool = ctx.enter_context(tc.tile_pool(name="wst", bufs=2))
        w_pool = ctx.enter_context(tc.tile_pool(name="w", bufs=1))
        # two weight buffer sets so the next matrix's fp16 cast never has to
        # wait for (or corrupt) the matmuls still reading the previous one
        w_A = [w_pool.tile([P, D], DT, name=f"wa{e}") for e in range(EC)]
        w_B = [w_pool.tile([P, D], DT, name=f"wb{e}") for e in range(EC)]
        e_pool = ctx.enter_context(tc.tile_pool(name="e", bufs=2))
        et_pool = ctx.enter_context(tc.tile_pool(name="et", bufs=2))
        r_pool = ctx.enter_context(tc.tile_pool(name="r", bufs=4))
        os_pool = ctx.enter_context(tc.tile_pool(name="os", bufs=2))

        def emit_body():
            def load_w(w_dram, w_sb):
                # DMA + immediate fp16 cast, half-e-chunk at a time
                for e in range(EC):
                    for h in range(2):
                        wst = wst_pool.tile([P, D // 2], F32, tag="wst")
                        nc.sync.dma_start(
                            wst[:], w_dram[e * P:(e + 1) * P,
                                           h * (D // 2):(h + 1) * (D // 2)])
                        nc.vector.tensor_copy(
                            w_sb[e][:, h * (D // 2):(h + 1) * (D // 2)],
                            wst[:])

            # qw first so P_Q can start early on a cold pipeline
            load_w(qw_d, w_A)

            # ---- P_X: x -> fp16 (GpSimd) -> xt via one batched XBAR
            # transpose per row-tile (3D out AP scatters all EC chunks)
            with tc.tile_pool(name="px_ps", bufs=4, space="PSUM") \
                    if not xt_dma else ExitStack() as px_pool:
                for ct in range(RB):
                    xst = xst_pool.tile([P, E], F32, tag="xst")
                    nc.sync.dma_start(xst[:], x_d[ct * P:(ct + 1) * P, :])
                    xh = xh_pool.tile([P, E], DT, tag="xh")
                    nc.gpsimd.tensor_copy(xh[:], xst[:])
                    if xt_dma:
                        out3d = xt_all[:, :].rearrange(
                            "p (e c) -> p e c", e=EC)[:, :, ct * P:(ct + 1) * P]
                        nc.sync.dma_start(out3d, xh[:, :], transpose=True)
                    else:
                        for e in range(EC):
                            ps_x = px_pool.tile([P, P], DT, tag="px")
                            nc.tensor.transpose(
                                ps_x[:], xh[:, e * P:(e + 1) * P], ident_h[:])
                            nc.vector.tensor_copy(
                                xt_sl(e, ct * P, (ct + 1) * P), ps_x[:])

            # kw -> B set: cast can run immediately (P_Q reads the A set)
            load_w(kw_d, w_B)

            def proj_t(w_sb, bias_t, dest, ps_pool):
                for cc in range(NCC):
                    for dc in range(DC):
                        ps = ps_pool.tile([P, CC], F32, tag="ps")
                        for e in range(EC):
                            nc.tensor.matmul(
                                ps[:],
                                w_sb[e][:, dc * P:(dc + 1) * P],
                                xt_sl(e, cc * CC, (cc + 1) * CC),
                                start=(e == 0), stop=(e == EC - 1))
                        nc.scalar.activation(
                            dest[dc][:, cc * CC:(cc + 1) * CC], ps[:],
                            AF.Identity, bias=bias_t[:, dc:dc + 1])

            with tc.tile_pool(name="pp_ps", bufs=6, space="PSUM") as ps_pool:
                proj_t(w_A, qb_t, qt_sb, ps_pool)
                # vw -> A set: DVE cast waits (WAR) for P_Q's last matmul
                load_w(vw_d, w_A)
                proj_t(w_B, kb_t, kt_sb, ps_pool)
                # ---- P_V: natural layout, bias via DVE add
                for ct in range(RB):
                    for dh in range(ND):
                        ps = ps_pool.tile([P, NJ], F32, tag="ps")
                        for e in range(EC):
                            nc.tensor.matmul(
                                ps[:], xt_sl(e, ct * P, (ct + 1) * P),
                                w_A[e][:, dh * NJ:(dh + 1) * NJ],
                                start=(e == 0), stop=(e == EC - 1))
                        nc.vector.tensor_add(
                            v_sb[ct][:, dh * NJ:(dh + 1) * NJ], ps[:],
                            vbb[:, dh * NJ:(dh + 1) * NJ])

            # ---- attention, software-pipelined by one row-block
            with tc.tile_pool(name="a_s", bufs=4, space="PSUM") as s_pool, \
                 tc.tile_pool(name="a_t", bufs=2, space="PSUM") as t_pool, \
                 tc.tile_pool(name="a_o", bufs=2, space="PSUM") as o_pool:

                def s_chain(i):
                    ncols = (i + 1) * P
                    njj = (ncols + NJ - 1) // NJ
                    etile = e_pool.tile([P, C], DT, tag="E")
                    acc = r_pool.tile([P, NCC], F32, tag="acc")
                    for jj in range(njj):
                        n = min(NJ, ncols - jj * NJ)
                        ps_s = s_pool.tile([P, NJ], F32, tag="s")
                        for d in range(DC):
                            nc.tensor.matmul(
                                ps_s[:, :n],
                                qt_sb[d][:, i * P:(i + 1) * P],
                                kt_sb[d][:, jj * NJ:jj * NJ + n],
                                start=(d == 0), stop=(d == DC - 1))
                        if jj == njj - 1:
                            dcol = n - P
                            nc.vector.tensor_add(ps_s[:, dcol:dcol + P],
                                                 ps_s[:, dcol:dcol + P],
                                                 cmask[:])
                        nc.scalar.activation(
                            etile[:, jj * NJ:jj * NJ + n], ps_s[:, :n],
                            AF.Exp, scale=scale,
                            accum_out=acc[:, jj:jj + 1])
                    return etile, acc

                def o_chain(k, etile, acc):
                    ncols = (k + 1) * P
                    njj = (ncols + NJ - 1) // NJ
                    rs = r_pool.tile([P, 1], F32, tag="rs")
                    nc.vector.reduce_sum(rs[:], acc[:, :njj],
                                         axis=mybir.AxisListType.X)
                    rinv = r_pool.tile([P, 1], F32, tag="rinv")
                    nc.vector.reciprocal(rinv[:], rs[:])

                    ettile = et_pool.tile([P, C], DT, tag="ET")
                    if et_dma:
                        for b in range(k + 1):
                            nc.scalar.dma_start(
                                ettile[:, b * P:(b + 1) * P],
                                etile[:, b * P:(b + 1) * P], transpose=True)
                    else:
                        for jj in range(njj):
                            n = min(NJ, ncols - jj * NJ)
                            ps_t = t_pool.tile([P, NJ], DT, tag="t")
                            for b in range(n // P):
                                nc.tensor.transpose(
                                    ps_t[:, b * P:(b + 1) * P],
                                    etile[:, jj * NJ + b * P:
                                          jj * NJ + (b + 1) * P],
                                    ident_h[:])
                            nc.vector.tensor_copy(
                                ettile[:, jj * NJ:jj * NJ + n], ps_t[:, :n])

                    for dh in range(ND):
                        ps_o = o_pool.tile([P, NJ], F32, tag="o")
                        for j in range(k + 1):
                            nc.tensor.matmul(
                                ps_o[:],
                                ettile[:, j * P:(j + 1) * P],
                                v_sb[j][:, dh * NJ:(dh + 1) * NJ],
                                start=(j == 0), stop=(j == k))
                        outst = os_pool.tile([P, NJ], F32, tag="os")
                        nc.vector.tensor_scalar_mul(outst[:], ps_o[:],
                                                    rinv[:])
                        nc.scalar.dma_start(
                            out_d[k * P:(k + 1) * P,
                                  dh * NJ:(dh + 1) * NJ], outst[:])

                prev = None
                for i in range(RB):
                    cur = s_chain(i)
                    if prev is not None:
                        o_chain(i - 1, *prev)
                    prev = cur
                o_chain(RB - 1, *prev)

        if loop > 1:
            with tc.For_i(0, loop // unroll, 1):
                for _u in range(unroll):
                    emit_body()
        else:
            emit_body()

    nc.compile()
    return nc


_CACHE = {}


def _built(C=2048, E=1024, D=1024, n_cores=8, loop=1, dt="fp16", unroll=1):
    xt_dma = os.environ.get("K_XT_DMA", "1") == "1"
    et_dma = os.environ.get("K_ET_DMA", "1") == "1"
    key = (C, E, D, n_cores, loop, dt, unroll, xt_dma, et_dma)
    if key not in _CACHE:
        _CACHE[key] = build(C, E, D, n_cores, loop, dt, unroll,
                            xt_dma=xt_dma, et_dma=et_dma)
    return _CACHE[key]


def _executable(C=2048, E=1024, D=1024, n_cores=8, loop=1, dt="fp16",
                unroll=1):
    """Cached jitted SPMD executable for the built Bass module."""
    key = ("exec", C, E, D, n_cores, loop, dt, unroll)
    if key in _CACHE:
        return _CACHE[key]
    import jax
    from jax.sharding import Mesh, PartitionSpec
    from jax.experimental.shard_map import shard_map
    from concourse import bass2jax, mybir as _mybir

    nc = _built(C, E, D, n_cores, loop, dt, unroll)
    bass2jax.install_neuronx_cc_hook()

    partition_name = (nc.partition_id_tensor.name
                      if nc.partition_id_tensor else None)
    in_names, out_names, out_avals, zero_outs = [], [], [], []
    for alloc in nc.m.functions[0].allocations:
        if not isinstance(alloc, _mybir.MemoryLocationSet):
            continue
        name = alloc.memorylocations[0].name
        if alloc.kind == "ExternalInput":
            if name != partition_name:
                in_names.append(name)
        elif alloc.kind == "ExternalOutput":
            out_names.append(name)
            shape = tuple(alloc.tensor_shape)
            dtype = _mybir.dt.np(alloc.dtype)
            out_avals.append(jax.core.ShapedArray(shape, dtype))
            zero_outs.append(np.zeros(shape, dtype))
    n_params = len(in_names)
    all_names = in_names + out_names
    if partition_name is not None:
        all_names = all_names + [partition_name]

    def _body(*args):
        operands = list(args)
        if partition_name is not None:
            operands.append(bass2jax.partition_id_tensor())
        outs = bass2jax._bass_exec_p.bind(
            *operands,
            out_avals=tuple(out_avals),
            in_names=tuple(all_names),
            out_names=tuple(out_names),
            lowering_input_output_aliases=(),
            sim_require_finite=True,
            sim_require_nnan=True,
            nc=nc,
        )
        return tuple(outs)

    devices = jax.devices()[:n_cores]
    mesh = Mesh(np.asarray(devices), ("core",))
    n_outs = len(out_names)
    sharded = jax.jit(
        shard_map(_body, mesh=mesh,
                  in_specs=(PartitionSpec("core"),) * (n_params + n_outs),
                  out_specs=(PartitionSpec("core"),) * n_outs,
                  check_rep=False),
        donate_argnums=tuple(range(n_params, n_params + n_outs)),
        keep_unused=True,
    )
    res = dict(fn=sharded, in_names=in_names, out_names=out_names,
               out_avals=out_avals, zero_outs=zero_outs, mesh=mesh,
               n_cores=n_cores)
    _CACHE[key] = res
    return res


def run(inputs, C=2048, E=1024, D=1024, n_cores=8, dt="fp16"):
    ex = _executable(C, E, D, n_cores, 1, dt)
    B = inputs["x"].shape[0]
    assert B == n_cores
    f = lambda a: np.ascontiguousarray(np.asarray(a, dtype=np.float32))
    shared = {k: f(inputs[k]) for k in ("Qw", "Qb", "Kw", "Kb", "Vw", "Vb")}
    x = f(inputs["x"])
    per_core = [dict(x=x[b], **shared) for b in range(B)]
    concat_in = [
        np.concatenate([per_core[c][n] for c in range(n_cores)], axis=0)
        for n in ex["in_names"]
    ]
    concat_zeros = [
        np.zeros((n_cores * z.shape[0], *z.shape[1:]), z.dtype)
        for z in ex["zero_outs"]
    ]
    out_arrs = ex["fn"](*concat_in, *concat_zeros)
    i = ex["out_names"].index("out")
    out = np.asarray(out_arrs[i]).reshape(n_cores, *ex["out_avals"][i].shape)
    return out


def kernel(**inputs) -> np.ndarray:
    return run(inputs)


# revision 19
# speedup vs baseline: 1.7050x; 1.2849x over previous
"""Causal single-head attention on 8 trn2 NeuronCores, data-parallel over batch.

Per core (one batch element, C=2048 ctx, E=1024 emb, D=1024 query_dim):

  P_X: DMA x row-tiles (fp32), cast to fp16 on GpSimd, DMA-XBAR-transpose
       into resident xt [e=128, C] fp16 tiles.  Zero PE cost.
  P_Q/P_K: qT/kT = (W^T @ x^T) + b via PE matmuls (N=512 moving), bias
       fused in the scalar-engine PSUM->SBUF copy.  Resident fp16.
  P_V: v = x @ Vw natural layout; bias added by DVE (broadcast tile vbb)
       during the PSUM->SBUF copy.  Resident fp16.
  Attention (software-pipelined by one row-block):
       S_i: scores for 128-query block i (PE, causal chunks of 512),
            additive mask on the diagonal tile (DVE), exp on ACT with
            fused row-sum accumulation.
       O_{i-1}: E^T via DMA-XBAR-transpose (no PE), AV accumulation on
            PE, 1/rowsum scale on DVE, DMA out.
       PE stream: S_0 S_1 O_0 S_2 O_1 ... S_15 O_14 O_15 -- the exp and
       transposes of block i hide under S_{i+1}/O_{i-1} PE work.

Loop mode (benchmarking): `loop` bodies total, emitted as For_i over
loop/unroll trips with `unroll` bodies per trip.  Unrolled bodies
overlap point-to-point (DMA/cast prefetch of body u+1 during body u's
attention); the all-engine For_i barrier is paid once per trip.
"""

import os
import sys

for _p in ("/opt/trn_rl_repo", "/root/.axon_site/_ro/trn_rl_repo"):
    if os.path.isdir(_p) and _p not in sys.path:
        sys.path.insert(0, _p)

from contextlib import ExitStack

import numpy as np

import concourse.bass as bass
import concourse.tile as tile
from concourse import bacc, mybir
from concourse.masks import make_causal_mask, make_identity

F32 = mybir.dt.float32
AF = mybir.ActivationFunctionType
DTYPES = {"fp16": mybir.dt.float16, "bf16": mybir.dt.bfloat16}

P = 128


def build(C=2048, E=1024, D=1024, n_cores=8, loop=1, dt="fp16", unroll=1,
          xt_dma=True, et_dma=False):
    DT = DTYPES[dt]
    CC = 512            # c-chunk width for projection passes
    NJ = 512            # matmul moving-dim width
    NCC = C // CC
    EC = E // P         # contraction chunks for projections
    DC = D // P
    RB = C // P         # number of 128-row blocks
    ND = D // NJ
    scale = float(D) ** -0.5

    assert loop % unroll == 0 or loop == 1
    nc = bacc.Bacc("TRN2", target_bir_lowering=False, debug=False,
                   num_devices=n_cores)
    x_d = nc.dram_tensor("x", [C, E], F32, kind="ExternalInput").ap()
    qw_d = nc.dram_tensor("Qw", [E, D], F32, kind="ExternalInput").ap()
    qb_d = nc.dram_tensor("Qb", [D], F32, kind="ExternalInput").ap()
    kw_d = nc.dram_tensor("Kw", [E, D], F32, kind="ExternalInput").ap()
    kb_d = nc.dram_tensor("Kb", [D], F32, kind="ExternalInput").ap()
    vw_d = nc.dram_tensor("Vw", [E, D], F32, kind="ExternalInput").ap()
    vb_d = nc.dram_tensor("Vb", [D], F32, kind="ExternalInput").ap()
    out_d = nc.dram_tensor("out", [C, D], F32, kind="ExternalOutput").ap()

    with tile.TileContext(nc) as tc, ExitStack() as ctx:
        # ---- loop-invariant constants (outside the For_i loop)
        const_pool = ctx.enter_context(tc.tile_pool(name="const", bufs=1))
        ident_f = const_pool.tile([P, P], F32, name="ident_f")
        make_identity(nc, ident_f)
        ident_h = const_pool.tile([P, P], DT, name="ident_h")
        nc.vector.tensor_copy(ident_h[:], ident_f[:])
        cmask = const_pool.tile([P, P], F32, name="cmask")
        make_causal_mask(nc, cmask, mask_val=-1e9)
        qb_t = const_pool.tile([P, DC], F32, name="qb_t")
        nc.sync.dma_start(qb_t[:], qb_d.rearrange("(c p) -> p c", p=P))
        kb_t = const_pool.tile([P, DC], F32, name="kb_t")
        nc.sync.dma_start(kb_t[:], kb_d.rearrange("(c p) -> p c", p=P))
        # V bias broadcast to all partitions (DVE adds it during P_V copy)
        vb_f = const_pool.tile([1, D], F32, name="vb_f")
        nc.sync.dma_start(vb_f[:], vb_d[None, :])
        vbb = const_pool.tile([P, D], F32, name="vbb")
        nc.gpsimd.partition_broadcast(vbb[:], vb_f[:], channels=P)

        # ---- persistent per-body tensors (shared across unrolled bodies;
        # WAR deps give natural cross-body pipelining)
        xt_pool = ctx.enter_context(tc.tile_pool(name="xt", bufs=1))
        qt_pool = ctx.enter_context(tc.tile_pool(name="qt", bufs=1))
        kt_pool = ctx.enter_context(tc.tile_pool(name="kt", bufs=1))
        v_pool = ctx.enter_context(tc.tile_pool(name="v", bufs=1))
        # xt is split into NCC tiles (one per 512-column group) so P_Q's
        # dependency on it is per-group, not whole-tensor; within a tile,
        # chunk e lives at cols [e*CC, (e+1)*CC).  One XBAR transpose per
        # x row-tile scatters all EC chunks via a 3D out AP.
        xt_cc = [xt_pool.tile([P, EC * CC], DT, name=f"xt{g}")
                 for g in range(NCC)]

        def xt_sl(e, lo, hi):
            g, lo_g = lo // CC, lo % CC
            assert hi - lo <= CC and lo_g + (hi - lo) <= CC
            return xt_cc[g][:, e * CC + lo_g:e * CC + lo_g + (hi - lo)]

        qt_sb = [qt_pool.tile([P, C], DT, name=f"qt{d}") for d in range(DC)]
        kt_sb = [kt_pool.tile([P, C], DT, name=f"kt{d}") for d in range(DC)]
        v_sb = [v_pool.tile([P, D], DT, name=f"v{i}") for i in range(RB)]

        # staging pools, persistent so WAR rotation spans bodies
        xst_pool = ctx.enter_context(tc.tile_pool(name="xst", bufs=2))
        xh_pool = ctx.enter_context(tc.tile_pool(name="xh", bufs=2))
        wst_pool = ctx.enter_context(tc.tile_pool(name="wst", bufs=2))
        w_pool = ctx.enter_context(tc.tile_pool(name="w", bufs=1))
        # two weight buffer sets so the next matrix's fp16 cast never has to
        # wait for (or corrupt) the matmuls still reading the previous one
        w_A = [w_pool.tile([P, D], DT, name=f"wa{e}") for e in range(EC)]
        w_B = [w_pool.tile([P, D], DT, name=f"wb{e}") for e in range(EC)]
        e_pool = ctx.enter_context(tc.tile_pool(name="e", bufs=2))
        et_pool = ctx.enter_context(tc.tile_pool(name="et", bufs=2))
        r_pool = ctx.enter_context(tc.tile_pool(name="r", bufs=4))
        os_pool = ctx.enter_context(tc.tile_pool(name="os", bufs=2))

        def load_w(w_dram, w_sb):
            # DMA + fp16 cast (GpSimd: keeps DVE/ACT free and lets next-body
            # prefetch run during this body's attention), half-chunks
            for e in range(EC):
                for h in range(2):
                    wst = wst_pool.tile([P, D // 2], F32, tag="wst")
                    nc.sync.dma_start(
                        wst[:], w_dram[e * P:(e + 1) * P,
                                       h * (D // 2):(h + 1) * (D // 2)])
                    nc.gpsimd.tensor_copy(
                        w_sb[e][:, h * (D // 2):(h + 1) * (D // 2)],
                        wst[:])

        def emit_loads():
            """DMA + cast + transpose of all inputs for one body.  Emitted
            one body AHEAD of the compute that consumes it (including
            across the For_i back-edge) so a body's compute never waits
            on input staging."""
            load_w(qw_d, w_A)
            # ---- P_X: x -> fp16 (GpSimd) -> xt via one batched XBAR
            # transpose per row-tile (3D out AP scatters all EC chunks)
            for ct in range(RB):
                xst = xst_pool.tile([P, E], F32, tag="xst")
                nc.sync.dma_start(xst[:], x_d[ct * P:(ct + 1) * P, :])
                xh = xh_pool.tile([P, E], DT, tag="xh")
                nc.gpsimd.tensor_copy(xh[:], xst[:])
                assert xt_dma
                g = ct // (CC // P)
                out3d = xt_cc[g][:, :].rearrange(
                    "p (e c) -> p e c",
                    e=EC)[:, :, (ct % (CC // P)) * P:
                          ((ct % (CC // P)) + 1) * P]
                nc.sync.dma_start(out3d, xh[:, :], transpose=True)
            # kw -> B set: cast can run immediately (P_Q reads the A set)
            load_w(kw_d, w_B)

        def emit_compute():
            def proj_t(w_sb, bias_t, dest, ps_pool):
                for cc in range(NCC):
                    for dc in range(DC):
                        ps = ps_pool.tile([P, CC], F32, tag="ps")
                        for e in range(EC):
                            nc.tensor.matmul(
                                ps[:],
                                w_sb[e][:, dc * P:(dc + 1) * P],
                                xt_sl(e, cc * CC, (cc + 1) * CC),
                                start=(e == 0), stop=(e == EC - 1))
                        nc.scalar.activation(
                            dest[dc][:, cc * CC:(cc + 1) * CC], ps[:],
                            AF.Identity, bias=bias_t[:, dc:dc + 1])

            with tc.tile_pool(name="pp_ps", bufs=6, space="PSUM") as ps_pool:
                proj_t(w_A, qb_t, qt_sb, ps_pool)
                # vw -> A set: GpSimd cast waits (WAR) for P_Q's last matmul
                load_w(vw_d, w_A)
                proj_t(w_B, kb_t, kt_sb, ps_pool)
                # ---- P_V: natural layout, bias via DVE add
                for ct in range(RB):
                    for dh in range(ND):
                        ps = ps_pool.tile([P, NJ], F32, tag="ps")
                        for e in range(EC):
                            nc.tensor.matmul(
                                ps[:], xt_sl(e, ct * P, (ct + 1) * P),
                                w_A[e][:, dh * NJ:(dh + 1) * NJ],
                                start=(e == 0), stop=(e == EC - 1))
                        nc.vector.tensor_add(
                            v_sb[ct][:, dh * NJ:(dh + 1) * NJ], ps[:],
                            vbb[:, dh * NJ:(dh + 1) * NJ])

            # ---- attention, software-pipelined by one row-block
            with tc.tile_pool(name="a_s", bufs=4, space="PSUM") as s_pool, \
                 tc.tile_pool(name="a_t", bufs=2, space="PSUM") as t_pool, \
                 tc.tile_pool(name="a_o", bufs=2, space="PSUM") as o_pool:

                def s_chain(i):
                    ncols = (i + 1) * P
                    njj = (ncols + NJ - 1) // NJ
                    etile = e_pool.tile([P, C], DT, tag="E")
                    acc = r_pool.tile([P, NCC], F32, tag="acc")
                    for jj in range(njj):
                        n = min(NJ, ncols - jj * NJ)
                        ps_s = s_pool.tile([P, NJ], F32, tag="s")
                        for d in range(DC):
                            nc.tensor.matmul(
                                ps_s[:, :n],
                                qt_sb[d][:, i * P:(i + 1) * P],
                                kt_sb[d][:, jj * NJ:jj * NJ + n],
                                start=(d == 0), stop=(d == DC - 1))
                        if jj == njj - 1:
                            dcol = n - P
                            nc.vector.tensor_add(ps_s[:, dcol:dcol + P],
                                                 ps_s[:, dcol:dcol + P],
                                                 cmask[:])
                        nc.scalar.activation(
                            etile[:, jj * NJ:jj * NJ + n], ps_s[:, :n],
                            AF.Exp, scale=scale,
                            accum_out=acc[:, jj:jj + 1])
                    return etile, acc

                def o_chain(k, etile, acc):
                    ncols = (k + 1) * P
                    njj = (ncols + NJ - 1) // NJ
                    rs = r_pool.tile([P, 1], F32, tag="rs")
                    nc.vector.reduce_sum(rs[:], acc[:, :njj],
                                         axis=mybir.AxisListType.X)
                    rinv = r_pool.tile([P, 1], F32, tag="rinv")
                    nc.vector.reciprocal(rinv[:], rs[:])

                    ettile = et_pool.tile([P, C], DT, tag="ET")
                    if et_dma:
                        for b in range(k + 1):
                            nc.scalar.dma_start(
                                ettile[:, b * P:(b + 1) * P],
                                etile[:, b * P:(b + 1) * P], transpose=True)
                    else:
                        for jj in range(njj):
                            n = min(NJ, ncols - jj * NJ)
                            ps_t = t_pool.tile([P, NJ], DT, tag="t")
                            for b in range(n // P):
                                nc.tensor.transpose(
                                    ps_t[:, b * P:(b + 1) * P],
                                    etile[:, jj * NJ + b * P:
                                          jj * NJ + (b + 1) * P],
                                    ident_h[:])
                            nc.vector.tensor_copy(
                                ettile[:, jj * NJ:jj * NJ + n], ps_t[:, :n])

                    for dh in range(ND):
                        ps_o = o_pool.tile([P, NJ], F32, tag="o")
                        for j in range(k + 1):
                            nc.tensor.matmul(
                                ps_o[:],
                                ettile[:, j * P:(j + 1) * P],
                                v_sb[j][:, dh * NJ:(dh + 1) * NJ],
                                start=(j == 0), stop=(j == k))
                        outst = os_pool.tile([P, NJ], F32, tag="os")
                        nc.vector.tensor_scalar_mul(outst[:], ps_o[:],
                                                    rinv[:])
                        nc.scalar.dma_start(
                            out_d[k * P:(k + 1) * P,
                                  dh * NJ:(dh + 1) * NJ], outst[:])

                prev = None
                for i in range(RB):
                    cur = s_chain(i)
                    if prev is not None:
                        o_chain(i - 1, *prev)
                    prev = cur
                o_chain(RB - 1, *prev)

        if loop > 1:
            # inputs for each body are staged one body ahead (the trip's
            # last emit_loads feeds the next trip's first body across the
            # For_i barrier), so compute never waits on input DMA
            emit_loads()
            with tc.For_i(0, loop // unroll, 1):
                for _u in range(unroll):
                    emit_compute()
                    emit_loads()
        else:
            emit_loads()
            emit_compute()

    nc.compile()
    return nc


_CACHE = {}


def _built(C=2048, E=1024, D=1024, n_cores=8, loop=1, dt="fp16", unroll=1):
    xt_dma = os.environ.get("K_XT_DMA", "1") == "1"
    et_dma = os.environ.get("K_ET_DMA", "0") == "1"
    key = (C, E, D, n_cores, loop, dt, unroll, xt_dma, et_dma)
    if key not in _CACHE:
        _CACHE[key] = build(C, E, D, n_cores, loop, dt, unroll,
                            xt_dma=xt_dma, et_dma=et_dma)
    return _CACHE[key]


def _executable(C=2048, E=1024, D=1024, n_cores=8, loop=1, dt="fp16",
                unroll=1):
    """Cached jitted SPMD executable for the built Bass module."""
    key = ("exec", C, E, D, n_cores, loop, dt, unroll)
    if key in _CACHE:
        return _CACHE[key]
    import jax
    from jax.sharding import Mesh, PartitionSpec
    from jax.experimental.shard_map import shard_map
    from concourse import bass2jax, mybir as _mybir

    nc = _built(C, E, D, n_cores, loop, dt, unroll)
    bass2jax.install_neuronx_cc_hook()

    partition_name = (nc.partition_id_tensor.name
                      if nc.partition_id_tensor else None)
    in_names, out_names, out_avals, zero_outs = [], [], [], []
    for alloc in nc.m.functions[0].allocations:
        if not isinstance(alloc, _mybir.MemoryLocationSet):
            continue
        name = alloc.memorylocations[0].name
        if alloc.kind == "ExternalInput":
            if name != partition_name:
                in_names.append(name)
        elif alloc.kind == "ExternalOutput":
            out_names.append(name)
            shape = tuple(alloc.tensor_shape)
            dtype = _mybir.dt.np(alloc.dtype)
            out_avals.append(jax.core.ShapedArray(shape, dtype))
            zero_outs.append(np.zeros(shape, dtype))
    n_params = len(in_names)
    all_names = in_names + out_names
    if partition_name is not None:
        all_names = all_names + [partition_name]

    def _body(*args):
        operands = list(args)
        if partition_name is not None:
            operands.append(bass2jax.partition_id_tensor())
        outs = bass2jax._bass_exec_p.bind(
            *operands,
            out_avals=tuple(out_avals),
            in_names=tuple(all_names),
            out_names=tuple(out_names),
            lowering_input_output_aliases=(),
            sim_require_finite=True,
            sim_require_nnan=True,
            nc=nc,
        )
        return tuple(outs)

    devices = jax.devices()[:n_cores]
    mesh = Mesh(np.asarray(devices), ("core",))
    n_outs = len(out_names)
    sharded = jax.jit(
        shard_map(_body, mesh=mesh,
                  in_specs=(PartitionSpec("core"),) * (n_params + n_outs),
                  out_specs=(PartitionSpec("core"),) * n_outs,
                  check_rep=False),
        donate_argnums=tuple(range(n_params, n_params + n_outs)),
        keep_unused=True,
    )
    res = dict(fn=sharded, in_names=in_names, out_names=out_names,
               out_avals=out_avals, zero_outs=zero_outs, mesh=mesh,
               n_cores=n_cores)
    _CACHE[key] = res
    return res


def run(inputs, C=2048, E=1024, D=1024, n_cores=8, dt="fp16"):
    ex = _executable(C, E, D, n_cores, 1, dt)
    B = inputs["x"].shape[0]
    assert B == n_cores
    f = lambda a: np.ascontiguousarray(np.asarray(a, dtype=np.float32))
    shared = {k: f(inputs[k]) for k in ("Qw", "Qb", "Kw", "Kb", "Vw", "Vb")}
    x = f(inputs["x"])
    per_core = [dict(x=x[b], **shared) for b in range(B)]
    concat_in = [
        np.concatenate([per_core[c][n] for c in range(n_cores)], axis=0)
        for n in ex["in_names"]
    ]
    concat_zeros = [
        np.zeros((n_cores * z.shape[0], *z.shape[1:]), z.dtype)
        for z in ex["zero_outs"]
    ]
    out_arrs = ex["fn"](*concat_in, *concat_zeros)
    i = ex["out_names"].index("out")
    out = np.asarray(out_arrs[i]).reshape(n_cores, *ex["out_avals"][i].shape)
    return out


def kernel(**inputs) -> np.ndarray:
    return run(inputs)
